# revision 46
# baseline (speedup 1.0000x reference)
"""GQA with sliding-window + ALiBi (reduces to banded causal attention) on 8 TRN2 cores.

Sharding: 8 cores = 2 batches x 4 kv-head groups. Each core computes, for its
(batch b, kv group gi): Q projection for its 4 query heads, K/V projection for
its 1 kv head, banded sliding-window attention (window 1024, causal), and a
partial row-parallel Wo matmul. Host sums the 4 partials per batch.

Math notes (exact reductions of the reference):
- ALiBi bias is -clip(j-i,0)*slope: zero on all causal positions, nonzero only
  where the causal mask kills the score -> drop it entirely.
- The sliding mask adds +1.0 uniformly inside the window: softmax-invariant.
- Out-of-window/causal positions get -1e9 -> exp underflows to exactly 0.
- Scores are O(1), so softmax without max-subtraction is safe.

Implementation (v3):
- All matmul operands and DMA traffic in bf16 (PSUM accumulation stays fp32);
  rel-err gate is 2e-2, end-to-end error measures ~5e-3.
- Whole hsT slab persists in SBUF (32KB/partition bf16), loaded by a handful
  of large multi-tile DMAs (HWDGE overhead is per-instruction); the first
  chunk arrives in pieces interleaved with wq so projections start early.
- V is produced pre-transposed ([s,d] layout) straight from the projection.
- 128-wide q blocks; masking = post-exp multiply by 0/1 patterns on DVE
  (only two patterns: causal diag, window lower edge), keeping the PE free.
- softmax denominator: ones[128,128]@pt broadcast-accumulated into the av
  bank as a contiguous group after av closes (matmul start=True clears
  accumulation bits for the whole PSUM bank, so open groups never interleave
  with another group's start in the same bank); DVE reciprocal + multiply.
- Attention is software-pipelined (scores -> exp -> av/den lag queue, depth
  5) and iterated qb-outer so each Wo row-tile issues as soon as its q block
  completes; pending work for a q block is drained before its Wo reads ohT.
"""
import math
from contextlib import ExitStack

import numpy as np

import concourse.tile as tile
from concourse import bacc, mybir
from concourse.bass_utils import run_bass_kernel_spmd
from concourse.masks import make_identity

dt = mybir.dt

B, S, H = 2, 2048, 2048
NUM_HEADS, KV_HEADS, D = 16, 4, 128
WINDOW = 1024
GH = 4            # query heads per kv head (per core)
GD = GH * D       # 512: per-core slice of the hidden dim
SCALE = 1.0 / math.sqrt(D)
NB = S // 128     # 16 128-wide blocks along s
KT = H // 128     # 16 contraction tiles for projections

_nc_cache = None


def _build_nc(pipe_depth=5, main_bufs=6, wo_bufs=2, pt_bufs=10, osb_bufs=4, debug=0,
              pe_warmup=1):
    nc = bacc.Bacc()
    hsT = nc.declare_dram_parameter("hsT", [KT, 128, S], dt.bfloat16, isOutput=False)
    wq = nc.declare_dram_parameter("wq", [H, GD], dt.bfloat16, isOutput=False)
    wk = nc.declare_dram_parameter("wk", [H, D], dt.bfloat16, isOutput=False)
    wv = nc.declare_dram_parameter("wv", [H, D], dt.bfloat16, isOutput=False)
    wo = nc.declare_dram_parameter("wo", [GD, H], dt.bfloat16, isOutput=False)
    masks = nc.declare_dram_parameter("masks", [2, 128, 128], dt.bfloat16, isOutput=False)
    out = nc.declare_dram_parameter("out", [NB, 4, 128, 512], dt.bfloat16, isOutput=True)

    with tile.TileContext(nc) as tc, ExitStack() as ctx:
        consts = ctx.enter_context(tc.tile_pool(name="consts", bufs=1))
        wpool = ctx.enter_context(tc.tile_pool(name="wpool", bufs=1))
        big = ctx.enter_context(tc.tile_pool(name="big", bufs=1))
        ptp = ctx.enter_context(tc.tile_pool(name="ptp", bufs=pt_bufs))
        smalls = ctx.enter_context(tc.tile_pool(name="smalls", bufs=4))
        outp = ctx.enter_context(tc.tile_pool(name="outp", bufs=osb_bufs))
        psum = ctx.enter_context(tc.tile_pool(name="psum", bufs=main_bufs, space="PSUM"))
        wops_p = ctx.enter_context(tc.tile_pool(name="wops", bufs=wo_bufs, space="PSUM"))

        # constants
        ident32 = consts.tile([128, 128], dt.float32)
        make_identity(nc, ident32)
        ident = consts.tile([128, 128], dt.bfloat16)
        nc.vector.tensor_copy(ident, ident32)
        ones32 = consts.tile([128, 128], dt.float32)
        nc.vector.memset(ones32, 1.0)
        ones = consts.tile([128, 128], dt.bfloat16)
        nc.vector.tensor_copy(ones, ones32)
        # PE warm-up: keep the PE busy through the initial DMA window so the
        # p-state ramp completes before real matmuls, and preload the Exp
        # activation table so the first attention batch doesn't pay it.
        warm_ps = psum.tile([128, 512], dt.float32, tag="ps", name="warm")
        for i in range(pe_warmup):
            nc.tensor.matmul(warm_ps[:, 0:128], lhsT=ident, rhs=ones,
                             start=True, stop=True)
        warm_sb = smalls.tile([128, 128], dt.float32, tag="warm_sb")
        nc.scalar.activation(warm_sb, warm_ps[:, 0:128],
                             mybir.ActivationFunctionType.Exp, scale=0.0)

        # weights + the whole hsT slab persist in SBUF (all bf16).
        # Few BIG multi-tile DMAs (HWDGE queue overhead is per-instruction).
        wq_all = wpool.tile([128, KT * GD], dt.bfloat16, tag="wq", name="wq_all")
        wk_all = wpool.tile([128, KT * D], dt.bfloat16, tag="wk", name="wk_all")
        wv_all = wpool.tile([128, KT * D], dt.bfloat16, tag="wv", name="wv_all")
        hs_all = wpool.tile([128, KT * S], dt.bfloat16, tag="hs", name="hs_all")
        wq_t = [wq_all[:, t * GD:(t + 1) * GD] for t in range(KT)]
        wk_t = [wk_all[:, t * D:(t + 1) * D] for t in range(KT)]
        wv_t = [wv_all[:, t * D:(t + 1) * D] for t in range(KT)]
        hs_t = [hs_all[:, t * S:(t + 1) * S] for t in range(KT)]
        mask_t = []
        for i in range(2):
            mt = consts.tile([128, 128], dt.bfloat16, tag=f"mask{i}", name=f"mask{i}")
            mask_t.append(mt)
        wo_all = wpool.tile([128, 4 * H], dt.bfloat16, tag="wo", name="wo_all")
        wo_t = [wo_all[:, ct * H:(ct + 1) * H] for ct in range(4)]

        # persistent activations
        qT = [big.tile([128, S], dt.bfloat16, tag=f"qT{h}", name=f"qT{h}") for h in range(GH)]
        kT = big.tile([128, S], dt.bfloat16, tag="kT")
        v = big.tile([128, S], dt.bfloat16, tag="v")  # [s%128, (sblk, d)]
        ohT = [big.tile([128, S], dt.bfloat16, tag=f"ohT{h}", name=f"ohT{h}") for h in range(GH)]

        # ---- DMA issue: first chunk's operands first, then the rest ----
        # wq as two halves (t 0-7, 8-15) so the first q-group starts sooner;
        # hs chunk DMAs carry all 16 t-tiles' 512-column slices in one instr.
        # SBUF-side DMA APs need the partition dim outermost; DRAM side is
        # rearranged to the same p-outer element order.
        def dma_hs_chunk(ch, t0, t1):
            nc.sync.dma_start(
                out=hs_all.rearrange("p (t s) -> p t s", t=KT)[:, t0:t1, ch * 512:(ch + 1) * 512],
                in_=hsT.rearrange("t p s -> p t s")[:, t0:t1, ch * 512:(ch + 1) * 512])

        wq_sb = wq_all.rearrange("p (t n) -> p t n", t=KT)
        wq_dr = wq.rearrange("(t p) n -> p t n", t=KT)
        first_pieces = [(0, 2), (2, 4), (4, 8), (8, 12), (12, 16)]
        for t0, t1 in first_pieces:
            nc.sync.dma_start(out=wq_sb[:, t0:t1], in_=wq_dr[:, t0:t1])
            dma_hs_chunk(0, t0, t1)
        nc.sync.dma_start(out=wk_all.rearrange("p (t n) -> p t n", t=KT),
                          in_=wk.rearrange("(t p) n -> p t n", t=KT))
        nc.sync.dma_start(out=wv_all.rearrange("p (t n) -> p t n", t=KT),
                          in_=wv.rearrange("(t p) n -> p t n", t=KT))
        for ch in range(1, 4):
            dma_hs_chunk(ch, 0, 16)
        for i in range(2):
            nc.sync.dma_start(out=mask_t[i], in_=masks[i])
        nc.sync.dma_start(out=wo_all.rearrange("p (t n) -> p t n", t=4),
                          in_=wo.rearrange("(t p) n -> p t n", t=4))

        # ---- Phase 1: projections per 256-wide half-chunk (3 PSUM banks each) ----
        # bank qps2: [h0 | h1] halves; qps2b: [h2 | h3]; bank kv: [k | v0 | v1]
        # NOTE: matmul start=True clears accumulation state for the WHOLE PSUM
        # bank, so co-resident groups in one bank must be issued contiguously
        # (a group fully closes before the next group's start): t-inner loops.
        # Chunk 0 runs t-major with one bank per q head so the PE can consume
        # quarter-granularity DMA arrivals without inter-group hazards.
        q_ps0 = [psum.tile([128, 512], dt.float32, tag="ps", name=f"q0_{h}")
                 for h in range(GH)]
        for t in range(KT):
            for h in range(GH):
                nc.tensor.matmul(q_ps0[h], lhsT=wq_t[t][:, h * 128:(h + 1) * 128],
                                 rhs=hs_t[t][:, 0:512],
                                 start=(t == 0), stop=(t == KT - 1))
        kv0_k = psum.tile([128, 512], dt.float32, tag="ps", name="k0")
        for t in range(KT):
            nc.tensor.matmul(kv0_k, lhsT=wk_t[t], rhs=hs_t[t][:, 0:512],
                             start=(t == 0), stop=(t == KT - 1))
        kv0_v = psum.tile([128, 512], dt.float32, tag="ps", name="v0")
        for j in range(4):
            for t in range(KT):
                nc.tensor.matmul(kv0_v[:, j * 128:(j + 1) * 128],
                                 lhsT=hs_t[t][:, j * 128:(j + 1) * 128],
                                 rhs=wv_t[t], start=(t == 0), stop=(t == KT - 1))
        for h in range(2):
            nc.vector.tensor_copy(qT[h][:, 0:512], q_ps0[h])
            nc.scalar.copy(qT[h + 2][:, 0:512], q_ps0[h + 2])
        nc.vector.tensor_copy(kT[:, 0:512], kv0_k)
        nc.vector.tensor_copy(v[:, 0:512], kv0_v)

        for hc in range(2, 8):
            s0 = hc * 256
            qps_a = psum.tile([128, 512], dt.float32, tag="ps", name=f"qa{hc}")
            qps_b = psum.tile([128, 512], dt.float32, tag="ps", name=f"qb{hc}")
            kv_ps = psum.tile([128, 512], dt.float32, tag="ps", name=f"kv{hc}")
            for h in range(2):
                for t in range(KT):
                    nc.tensor.matmul(qps_a[:, h * 256:(h + 1) * 256],
                                     lhsT=wq_t[t][:, h * 128:(h + 1) * 128],
                                     rhs=hs_t[t][:, s0:s0 + 256],
                                     start=(t == 0), stop=(t == KT - 1))
            for h in range(2):
                for t in range(KT):
                    nc.tensor.matmul(qps_b[:, h * 256:(h + 1) * 256],
                                     lhsT=wq_t[t][:, (h + 2) * 128:(h + 3) * 128],
                                     rhs=hs_t[t][:, s0:s0 + 256],
                                     start=(t == 0), stop=(t == KT - 1))
            for t in range(KT):
                nc.tensor.matmul(kv_ps[:, 0:256], lhsT=wk_t[t],
                                 rhs=hs_t[t][:, s0:s0 + 256],
                                 start=(t == 0), stop=(t == KT - 1))
            for j in range(2):
                for t in range(KT):
                    nc.tensor.matmul(kv_ps[:, 256 + j * 128:256 + (j + 1) * 128],
                                     lhsT=hs_t[t][:, s0 + j * 128:s0 + (j + 1) * 128],
                                     rhs=wv_t[t], start=(t == 0), stop=(t == KT - 1))
            for h in range(2):
                nc.vector.tensor_copy(qT[h][:, s0:s0 + 256],
                                      qps_a[:, h * 256:(h + 1) * 256])
                nc.scalar.copy(qT[h + 2][:, s0:s0 + 256],
                               qps_b[:, h * 256:(h + 1) * 256])
            nc.vector.tensor_copy(kT[:, s0:s0 + 256], kv_ps[:, 0:256])
            # v blocks 2*hc, 2*hc+1 -> v[:, blk*128:(blk+1)*128]
            nc.vector.tensor_copy(v[:, s0:s0 + 256], kv_ps[:, 256:512])

        if debug == 1:
            # dump projections: out[0..3]=qT, out[4]=kT, out[5]=v
            for e in range(4):
                for h in range(GH):
                    nc.sync.dma_start(out=out[h, e], in_=qT[h][:, e * 512:(e + 1) * 512])
                nc.sync.dma_start(out=out[4, e], in_=kT[:, e * 512:(e + 1) * 512])
                nc.sync.dma_start(out=out[5, e], in_=v[:, e * 512:(e + 1) * 512])

        # ---- Phase 2+3: banded attention (qb-outer) + Wo row-tiles ----
        # per (h, qb): kjs = [max(0, qb-8) .. qb]; score blocks [128k x 128q]
        # accumulated transposed; exp batches of <=4 blocks per PSUM bank.
        pending = []   # (avden, pts, pt, kj_list, first, last, h, qb)

        def flush_one():
            # av accumulates alone as the bank's open group; den is issued as
            # one contiguous group into the same bank only after av has closed
            # (a start=True clears accumulation bits bank-wide).
            avden, pts, pt, kjl, first, last, h, qb = pending.pop(0)
            n = len(kjl)
            for i, kj in enumerate(kjl):
                nc.tensor.matmul(avden[:, 0:128], lhsT=v[:, kj * 128:(kj + 1) * 128],
                                 rhs=pt[:, i * 128:(i + 1) * 128],
                                 start=(first and i == 0), stop=(last and i == n - 1))
            if last:
                nkj = sum(len(bk) for _, bk in pts)
                d = 0
                for ptt, bk in pts:
                    for i in range(len(bk)):
                        nc.tensor.matmul(avden[:, 128:256], lhsT=ones,
                                         rhs=ptt[:, i * 128:(i + 1) * 128],
                                         start=(d == 0), stop=(d == nkj - 1))
                        d += 1
                rcb = smalls.tile([128, 128], dt.float32, tag="rcb")
                with nc.allow_low_precision(reason="fp32 reciprocal, full precision"):
                    nc.vector.reciprocal(rcb, avden[:, 128:256])
                nc.vector.tensor_mul(ohT[h][:, qb * 128:(qb + 1) * 128],
                                     avden[:, 0:128], rcb)

        for qb in range(NB if debug != 1 else 0):
            for h in range(GH):
                kjs = list(range(max(0, qb - 8), qb + 1))
                avden = psum.tile([128, 512], dt.float32, tag="ps", name=f"ad{qb}_{h}")
                qs = qT[h][:, qb * 128:(qb + 1) * 128]
                pts = []
                for bi in range(0, len(kjs), 4):
                    bk = kjs[bi:bi + 4]
                    sps = psum.tile([128, 512], dt.float32, tag="ps")
                    for i, kj in enumerate(bk):
                        nc.tensor.matmul(sps[:, i * 128:(i + 1) * 128],
                                         lhsT=kT[:, kj * 128:(kj + 1) * 128],
                                         rhs=qs, start=True, stop=True)
                    pt = ptp.tile([128, 512], dt.bfloat16, tag="pt")
                    nc.scalar.activation(pt[:, :128 * len(bk)], sps[:, :128 * len(bk)],
                                         mybir.ActivationFunctionType.Exp, scale=SCALE)
                    # mask by zeroing exp weights (unmasked exp can't overflow:
                    # |score*scale| <= sqrt(128)*|q||k|*scale ~ O(12))
                    for i, kj in enumerate(bk):
                        mi = 0 if kj == qb else (1 if kj == qb - 8 else None)
                        if mi is not None:
                            nc.vector.tensor_mul(pt[:, i * 128:(i + 1) * 128],
                                                 pt[:, i * 128:(i + 1) * 128],
                                                 mask_t[mi])
                    pts.append((pt, bk))
                    pending.append((avden, pts, pt, bk, bi == 0, bi + 4 >= len(kjs), h, qb))
                    while len(pending) > pipe_depth:
                        flush_one()
            # Wo row-tile st=qb-1; first drain any pending work for that qb so
            # its divides are issued before the Wo matmuls read ohT
            if qb >= 1:
                while any(item[7] == qb - 1 for item in pending):
                    flush_one()
                emit_wo(nc, wops_p, outp, ohT, wo_t, out, qb - 1)
        while pending:
            flush_one()
        if debug != 1:
            emit_wo(nc, wops_p, outp, ohT, wo_t, out, NB - 1, split_dma=False)

    nc.compile()
    return nc


def emit_wo(nc, wops_p, outp, ohT, wo_t, out, st, split_dma=False):
    osb = outp.tile([128, 4 * 512], dt.bfloat16, tag="osb")
    for e in range(4):
        wops = wops_p.tile([128, 512], dt.float32, tag="wo")
        for ct in range(4):
            nc.tensor.matmul(wops, lhsT=ohT[ct][:, st * 128:(st + 1) * 128],
                             rhs=wo_t[ct][:, e * 512:(e + 1) * 512],
                             start=(ct == 0), stop=(ct == 3))
        nc.vector.tensor_copy(osb[:, e * 512:(e + 1) * 512], wops)
        if split_dma:
            nc.sync.dma_start(out=out[st, e], in_=osb[:, e * 512:(e + 1) * 512])
    if not split_dma:
        nc.sync.dma_start(out=out[st].rearrange("e p n -> p e n"), in_=osb)


def _build_masks():
    kk = np.arange(128)[:, None]
    qq = np.arange(128)[None, :]
    diag = (kk <= qq).astype(np.float32)   # causal within diag block
    edge = (kk >= qq).astype(np.float32)   # window lower edge
    return np.stack([diag, edge])


def kernel(hidden_states, Wq, Wk, Wv, Wo):
    global _nc_cache
    if _nc_cache is None:
        _nc_cache = _build_nc()
    nc = _nc_cache

    bf16 = dt.np(dt.bfloat16)
    masks = _build_masks().astype(bf16)
    hsT = []
    for b in range(B):
        ht = np.ascontiguousarray(hidden_states[b].T.astype(bf16))    # [H, S]
        hsT.append(np.ascontiguousarray(ht.reshape(KT, 128, S)))
    in_maps = []
    for b in range(B):
        for gi in range(KV_HEADS):
            in_maps.append({
                "hsT": hsT[b],
                "wq": np.ascontiguousarray(Wq[:, gi * GD:(gi + 1) * GD].astype(bf16)),
                "wk": np.ascontiguousarray(Wk[:, gi * D:(gi + 1) * D].astype(bf16)),
                "wv": np.ascontiguousarray(Wv[:, gi * D:(gi + 1) * D].astype(bf16)),
                "wo": np.ascontiguousarray(Wo[gi * GD:(gi + 1) * GD, :].astype(bf16)),
                "masks": masks,
            })
    res = run_bass_kernel_spmd(nc, in_maps, list(range(8)))
    out = np.zeros((B, S, H), np.float32)
    for b in range(B):
        acc = None
        for gi in range(KV_HEADS):
            o = res.results[b * KV_HEADS + gi]["out"].astype(np.float32)
            acc = o if acc is None else acc + o
        out[b] = acc.transpose(0, 2, 1, 3).reshape(S, H)              # [16,4,128,512] -> [S,H]
    return out


# revision 48
# speedup vs baseline: 1.0007x; 1.0007x over previous
"""GQA with sliding-window + ALiBi (reduces to banded causal attention) on 8 TRN2 cores.

Sharding: 8 cores = 2 batches x 4 kv-head groups. Each core computes, for its
(batch b, kv group gi): Q projection for its 4 query heads, K/V projection for
its 1 kv head, banded sliding-window attention (window 1024, causal), and a
partial row-parallel Wo matmul. Host sums the 4 partials per batch.

Math notes (exact reductions of the reference):
- ALiBi bias is -clip(j-i,0)*slope: zero on all causal positions, nonzero only
  where the causal mask kills the score -> drop it entirely.
- The sliding mask adds +1.0 uniformly inside the window: softmax-invariant.
- Out-of-window/causal positions get -1e9 -> exp underflows to exactly 0.
- Scores are O(1), so softmax without max-subtraction is safe.

Implementation (v3):
- All matmul operands and DMA traffic in bf16 (PSUM accumulation stays fp32);
  rel-err gate is 2e-2, end-to-end error measures ~5e-3.
- Whole hsT slab persists in SBUF (32KB/partition bf16), loaded by a handful
  of large multi-tile DMAs (HWDGE overhead is per-instruction); the first
  chunk arrives in pieces interleaved with wq so projections start early.
- V is produced pre-transposed ([s,d] layout) straight from the projection.
- 128-wide q blocks; masking = post-exp multiply by 0/1 patterns on DVE
  (only two patterns: causal diag, window lower edge), keeping the PE free.
- softmax denominator: ones[128,128]@pt broadcast-accumulated into the av
  bank as a contiguous group after av closes (matmul start=True clears
  accumulation bits for the whole PSUM bank, so open groups never interleave
  with another group's start in the same bank); DVE reciprocal + multiply.
- Attention is software-pipelined (scores -> exp -> av/den lag queue, depth
  5) and iterated qb-outer so each Wo row-tile issues as soon as its q block
  completes; pending work for a q block is drained before its Wo reads ohT.
"""
import math
from contextlib import ExitStack

import numpy as np

import concourse.tile as tile
from concourse import bacc, mybir
from concourse.bass_utils import run_bass_kernel_spmd
from concourse.masks import make_identity

dt = mybir.dt

B, S, H = 2, 2048, 2048
NUM_HEADS, KV_HEADS, D = 16, 4, 128
WINDOW = 1024
GH = 4            # query heads per kv head (per core)
GD = GH * D       # 512: per-core slice of the hidden dim
SCALE = 1.0 / math.sqrt(D)
NB = S // 128     # 16 128-wide blocks along s
KT = H // 128     # 16 contraction tiles for projections

_nc_cache = None


def _build_nc(pipe_depth=5, main_bufs=6, wo_bufs=2, pt_bufs=10, osb_bufs=4, debug=0,
              pe_warmup=1):
    nc = bacc.Bacc()
    hsT = nc.declare_dram_parameter("hsT", [KT, 128, S], dt.bfloat16, isOutput=False)
    # wq and hs chunk 0 interleaved per t: [wq_t | hs_t[:, 0:512]] -> [KT,128,1024]
    wqhs0 = nc.declare_dram_parameter("wqhs0", [KT, 128, 2 * GD], dt.bfloat16,
                                      isOutput=False)
    wk = nc.declare_dram_parameter("wk", [H, D], dt.bfloat16, isOutput=False)
    wv = nc.declare_dram_parameter("wv", [H, D], dt.bfloat16, isOutput=False)
    wo = nc.declare_dram_parameter("wo", [GD, H], dt.bfloat16, isOutput=False)
    masks = nc.declare_dram_parameter("masks", [2, 128, 128], dt.bfloat16, isOutput=False)
    out = nc.declare_dram_parameter("out", [NB, 4, 128, 512], dt.bfloat16, isOutput=True)

    with tile.TileContext(nc) as tc, ExitStack() as ctx:
        consts = ctx.enter_context(tc.tile_pool(name="consts", bufs=1))
        wpool = ctx.enter_context(tc.tile_pool(name="wpool", bufs=1))
        big = ctx.enter_context(tc.tile_pool(name="big", bufs=1))
        ptp = ctx.enter_context(tc.tile_pool(name="ptp", bufs=pt_bufs))
        smalls = ctx.enter_context(tc.tile_pool(name="smalls", bufs=4))
        outp = ctx.enter_context(tc.tile_pool(name="outp", bufs=osb_bufs))
        psum = ctx.enter_context(tc.tile_pool(name="psum", bufs=main_bufs, space="PSUM"))
        wops_p = ctx.enter_context(tc.tile_pool(name="wops", bufs=wo_bufs, space="PSUM"))

        # constants
        ident32 = consts.tile([128, 128], dt.float32)
        make_identity(nc, ident32)
        ident = consts.tile([128, 128], dt.bfloat16)
        nc.vector.tensor_copy(ident, ident32)
        ones32 = consts.tile([128, 128], dt.float32)
        nc.vector.memset(ones32, 1.0)
        ones = consts.tile([128, 128], dt.bfloat16)
        nc.vector.tensor_copy(ones, ones32)
        # PE warm-up: keep the PE busy through the initial DMA window so the
        # p-state ramp completes before real matmuls, and preload the Exp
        # activation table so the first attention batch doesn't pay it.
        warm_ps = psum.tile([128, 512], dt.float32, tag="ps", name="warm")
        for i in range(pe_warmup):
            nc.tensor.matmul(warm_ps[:, 0:128], lhsT=ident, rhs=ones,
                             start=True, stop=True)
        warm_sb = smalls.tile([128, 128], dt.float32, tag="warm_sb")
        nc.scalar.activation(warm_sb, warm_ps[:, 0:128],
                             mybir.ActivationFunctionType.Exp, scale=0.0)

        # weights + the whole hsT slab persist in SBUF (all bf16).
        # Few BIG multi-tile DMAs (HWDGE queue overhead is per-instruction).
        wqhs_all = wpool.tile([128, KT * 2 * GD], dt.bfloat16, tag="wqhs", name="wqhs_all")
        wk_all = wpool.tile([128, KT * D], dt.bfloat16, tag="wk", name="wk_all")
        wv_all = wpool.tile([128, KT * D], dt.bfloat16, tag="wv", name="wv_all")
        hs_all = wpool.tile([128, KT * S], dt.bfloat16, tag="hs", name="hs_all")
        wq_t = [wqhs_all[:, t * 2 * GD:t * 2 * GD + GD] for t in range(KT)]
        hs0_t = [wqhs_all[:, t * 2 * GD + GD:(t + 1) * 2 * GD] for t in range(KT)]
        wk_t = [wk_all[:, t * D:(t + 1) * D] for t in range(KT)]
        wv_t = [wv_all[:, t * D:(t + 1) * D] for t in range(KT)]
        hs_t = [hs_all[:, t * S:(t + 1) * S] for t in range(KT)]
        mask_t = []
        for i in range(2):
            mt = consts.tile([128, 128], dt.bfloat16, tag=f"mask{i}", name=f"mask{i}")
            mask_t.append(mt)
        wo_all = wpool.tile([128, 4 * H], dt.bfloat16, tag="wo", name="wo_all")
        wo_t = [wo_all[:, ct * H:(ct + 1) * H] for ct in range(4)]

        # persistent activations
        qT = [big.tile([128, S], dt.bfloat16, tag=f"qT{h}", name=f"qT{h}") for h in range(GH)]
        kT = big.tile([128, S], dt.bfloat16, tag="kT")
        v = big.tile([128, S], dt.bfloat16, tag="v")  # [s%128, (sblk, d)]
        ohT = [big.tile([128, S], dt.bfloat16, tag=f"ohT{h}", name=f"ohT{h}") for h in range(GH)]

        # ---- DMA issue: first chunk's operands first, then the rest ----
        # wq as two halves (t 0-7, 8-15) so the first q-group starts sooner;
        # hs chunk DMAs carry all 16 t-tiles' 512-column slices in one instr.
        # SBUF-side DMA APs need the partition dim outermost; DRAM side is
        # rearranged to the same p-outer element order.
        def dma_hs_chunk(ch, t0, t1):
            nc.sync.dma_start(
                out=hs_all.rearrange("p (t s) -> p t s", t=KT)[:, t0:t1, ch * 512:(ch + 1) * 512],
                in_=hsT.rearrange("t p s -> p t s")[:, t0:t1, ch * 512:(ch + 1) * 512])

        wqhs_sb = wqhs_all.rearrange("p (t n) -> p t n", t=KT)
        wqhs_dr = wqhs0.rearrange("t p n -> p t n")
        first_pieces = [(0, 2), (2, 4), (4, 8), (8, 12), (12, 16)]
        for t0, t1 in first_pieces:
            nc.sync.dma_start(out=wqhs_sb[:, t0:t1], in_=wqhs_dr[:, t0:t1])
        nc.sync.dma_start(out=wk_all.rearrange("p (t n) -> p t n", t=KT),
                          in_=wk.rearrange("(t p) n -> p t n", t=KT))
        nc.sync.dma_start(out=wv_all.rearrange("p (t n) -> p t n", t=KT),
                          in_=wv.rearrange("(t p) n -> p t n", t=KT))
        for ch in range(1, 4):
            dma_hs_chunk(ch, 0, 16)
        for i in range(2):
            nc.sync.dma_start(out=mask_t[i], in_=masks[i])
        nc.sync.dma_start(out=wo_all.rearrange("p (t n) -> p t n", t=4),
                          in_=wo.rearrange("(t p) n -> p t n", t=4))

        # ---- Phase 1: projections per 256-wide half-chunk (3 PSUM banks each) ----
        # bank qps2: [h0 | h1] halves; qps2b: [h2 | h3]; bank kv: [k | v0 | v1]
        # NOTE: matmul start=True clears accumulation state for the WHOLE PSUM
        # bank, so co-resident groups in one bank must be issued contiguously
        # (a group fully closes before the next group's start): t-inner loops.
        # Chunk 0 runs t-major with one bank per q head so the PE can consume
        # quarter-granularity DMA arrivals without inter-group hazards.
        q_ps0 = [psum.tile([128, 512], dt.float32, tag="ps", name=f"q0_{h}")
                 for h in range(GH)]
        for t in range(KT):
            for h in range(GH):
                nc.tensor.matmul(q_ps0[h], lhsT=wq_t[t][:, h * 128:(h + 1) * 128],
                                 rhs=hs0_t[t],
                                 start=(t == 0), stop=(t == KT - 1))
        kv0_k = psum.tile([128, 512], dt.float32, tag="ps", name="k0")
        for t in range(KT):
            nc.tensor.matmul(kv0_k, lhsT=wk_t[t], rhs=hs0_t[t],
                             start=(t == 0), stop=(t == KT - 1))
        kv0_v = psum.tile([128, 512], dt.float32, tag="ps", name="v0")
        for j in range(4):
            for t in range(KT):
                nc.tensor.matmul(kv0_v[:, j * 128:(j + 1) * 128],
                                 lhsT=hs0_t[t][:, j * 128:(j + 1) * 128],
                                 rhs=wv_t[t], start=(t == 0), stop=(t == KT - 1))
        for h in range(2):
            nc.vector.tensor_copy(qT[h][:, 0:512], q_ps0[h])
            nc.scalar.copy(qT[h + 2][:, 0:512], q_ps0[h + 2])
        nc.vector.tensor_copy(kT[:, 0:512], kv0_k)
        nc.vector.tensor_copy(v[:, 0:512], kv0_v)

        for hc in range(2, 8):
            s0 = hc * 256
            qps_a = psum.tile([128, 512], dt.float32, tag="ps", name=f"qa{hc}")
            qps_b = psum.tile([128, 512], dt.float32, tag="ps", name=f"qb{hc}")
            kv_ps = psum.tile([128, 512], dt.float32, tag="ps", name=f"kv{hc}")
            for h in range(2):
                for t in range(KT):
                    nc.tensor.matmul(qps_a[:, h * 256:(h + 1) * 256],
                                     lhsT=wq_t[t][:, h * 128:(h + 1) * 128],
                                     rhs=hs_t[t][:, s0:s0 + 256],
                                     start=(t == 0), stop=(t == KT - 1))
            for h in range(2):
                for t in range(KT):
                    nc.tensor.matmul(qps_b[:, h * 256:(h + 1) * 256],
                                     lhsT=wq_t[t][:, (h + 2) * 128:(h + 3) * 128],
                                     rhs=hs_t[t][:, s0:s0 + 256],
                                     start=(t == 0), stop=(t == KT - 1))
            for t in range(KT):
                nc.tensor.matmul(kv_ps[:, 0:256], lhsT=wk_t[t],
                                 rhs=hs_t[t][:, s0:s0 + 256],
                                 start=(t == 0), stop=(t == KT - 1))
            for j in range(2):
                for t in range(KT):
                    nc.tensor.matmul(kv_ps[:, 256 + j * 128:256 + (j + 1) * 128],
                                     lhsT=hs_t[t][:, s0 + j * 128:s0 + (j + 1) * 128],
                                     rhs=wv_t[t], start=(t == 0), stop=(t == KT - 1))
            for h in range(2):
                nc.vector.tensor_copy(qT[h][:, s0:s0 + 256],
                                      qps_a[:, h * 256:(h + 1) * 256])
                nc.scalar.copy(qT[h + 2][:, s0:s0 + 256],
                               qps_b[:, h * 256:(h + 1) * 256])
            nc.vector.tensor_copy(kT[:, s0:s0 + 256], kv_ps[:, 0:256])
            # v blocks 2*hc, 2*hc+1 -> v[:, blk*128:(blk+1)*128]
            nc.vector.tensor_copy(v[:, s0:s0 + 256], kv_ps[:, 256:512])

        if debug == 1:
            # dump projections: out[0..3]=qT, out[4]=kT, out[5]=v
            for e in range(4):
                for h in range(GH):
                    nc.sync.dma_start(out=out[h, e], in_=qT[h][:, e * 512:(e + 1) * 512])
                nc.sync.dma_start(out=out[4, e], in_=kT[:, e * 512:(e + 1) * 512])
                nc.sync.dma_start(out=out[5, e], in_=v[:, e * 512:(e + 1) * 512])

        # ---- Phase 2+3: banded attention (qb-outer) + Wo row-tiles ----
        # per (h, qb): kjs = [max(0, qb-8) .. qb]; score blocks [128k x 128q]
        # accumulated transposed; exp batches of <=4 blocks per PSUM bank.
        pending = []   # (avden, pts, pt, kj_list, first, last, h, qb)

        def flush_one():
            # av accumulates alone as the bank's open group; den is issued as
            # one contiguous group into the same bank only after av has closed
            # (a start=True clears accumulation bits bank-wide).
            avden, pts, pt, kjl, first, last, h, qb = pending.pop(0)
            n = len(kjl)
            for i, kj in enumerate(kjl):
                nc.tensor.matmul(avden[:, 0:128], lhsT=v[:, kj * 128:(kj + 1) * 128],
                                 rhs=pt[:, i * 128:(i + 1) * 128],
                                 start=(first and i == 0), stop=(last and i == n - 1))
            if last:
                nkj = sum(len(bk) for _, bk in pts)
                d = 0
                for ptt, bk in pts:
                    for i in range(len(bk)):
                        nc.tensor.matmul(avden[:, 128:256], lhsT=ones,
                                         rhs=ptt[:, i * 128:(i + 1) * 128],
                                         start=(d == 0), stop=(d == nkj - 1))
                        d += 1
                rcb = smalls.tile([128, 128], dt.float32, tag="rcb")
                with nc.allow_low_precision(reason="fp32 reciprocal, full precision"):
                    nc.vector.reciprocal(rcb, avden[:, 128:256])
                nc.vector.tensor_mul(ohT[h][:, qb * 128:(qb + 1) * 128],
                                     avden[:, 0:128], rcb)

        for qb in range(NB if debug != 1 else 0):
            for h in range(GH):
                kjs = list(range(max(0, qb - 8), qb + 1))
                avden = psum.tile([128, 512], dt.float32, tag="ps", name=f"ad{qb}_{h}")
                qs = qT[h][:, qb * 128:(qb + 1) * 128]
                pts = []
                for bi in range(0, len(kjs), 4):
                    bk = kjs[bi:bi + 4]
                    sps = psum.tile([128, 512], dt.float32, tag="ps")
                    for i, kj in enumerate(bk):
                        nc.tensor.matmul(sps[:, i * 128:(i + 1) * 128],
                                         lhsT=kT[:, kj * 128:(kj + 1) * 128],
                                         rhs=qs, start=True, stop=True)
                    pt = ptp.tile([128, 512], dt.bfloat16, tag="pt")
                    nc.scalar.activation(pt[:, :128 * len(bk)], sps[:, :128 * len(bk)],
                                         mybir.ActivationFunctionType.Exp, scale=SCALE)
                    # mask by zeroing exp weights (unmasked exp can't overflow:
                    # |score*scale| <= sqrt(128)*|q||k|*scale ~ O(12))
                    for i, kj in enumerate(bk):
                        mi = 0 if kj == qb else (1 if kj == qb - 8 else None)
                        if mi is not None:
                            nc.vector.tensor_mul(pt[:, i * 128:(i + 1) * 128],
                                                 pt[:, i * 128:(i + 1) * 128],
                                                 mask_t[mi])
                    pts.append((pt, bk))
                    pending.append((avden, pts, pt, bk, bi == 0, bi + 4 >= len(kjs), h, qb))
                    while len(pending) > pipe_depth:
                        flush_one()
            # Wo row-tile st=qb-1; first drain any pending work for that qb so
            # its divides are issued before the Wo matmuls read ohT
            if qb >= 1:
                while any(item[7] == qb - 1 for item in pending):
                    flush_one()
                emit_wo(nc, wops_p, outp, ohT, wo_t, out, qb - 1)
        while pending:
            flush_one()
        if debug != 1:
            emit_wo(nc, wops_p, outp, ohT, wo_t, out, NB - 1, split_dma=False)

    nc.compile()
    return nc


def emit_wo(nc, wops_p, outp, ohT, wo_t, out, st, split_dma=False):
    osb = outp.tile([128, 4 * 512], dt.bfloat16, tag="osb")
    for e in range(4):
        wops = wops_p.tile([128, 512], dt.float32, tag="wo")
        for ct in range(4):
            nc.tensor.matmul(wops, lhsT=ohT[ct][:, st * 128:(st + 1) * 128],
                             rhs=wo_t[ct][:, e * 512:(e + 1) * 512],
                             start=(ct == 0), stop=(ct == 3))
        nc.vector.tensor_copy(osb[:, e * 512:(e + 1) * 512], wops)
        if split_dma:
            nc.sync.dma_start(out=out[st, e], in_=osb[:, e * 512:(e + 1) * 512])
    if not split_dma:
        nc.sync.dma_start(out=out[st].rearrange("e p n -> p e n"), in_=osb)


def _build_masks():
    kk = np.arange(128)[:, None]
    qq = np.arange(128)[None, :]
    diag = (kk <= qq).astype(np.float32)   # causal within diag block
    edge = (kk >= qq).astype(np.float32)   # window lower edge
    return np.stack([diag, edge])


def kernel(hidden_states, Wq, Wk, Wv, Wo):
    global _nc_cache
    if _nc_cache is None:
        _nc_cache = _build_nc()
    nc = _nc_cache

    bf16 = dt.np(dt.bfloat16)
    masks = _build_masks().astype(bf16)
    hsT = []
    for b in range(B):
        ht = np.ascontiguousarray(hidden_states[b].T.astype(bf16))    # [H, S]
        hsT.append(np.ascontiguousarray(ht.reshape(KT, 128, S)))
    in_maps = []
    for b in range(B):
        for gi in range(KV_HEADS):
            wq_sl = Wq[:, gi * GD:(gi + 1) * GD].astype(bf16).reshape(KT, 128, GD)
            in_maps.append({
                "hsT": hsT[b],
                "wqhs0": np.ascontiguousarray(
                    np.concatenate([wq_sl, hsT[b][:, :, 0:GD]], axis=2)),
                "wk": np.ascontiguousarray(Wk[:, gi * D:(gi + 1) * D].astype(bf16)),
                "wv": np.ascontiguousarray(Wv[:, gi * D:(gi + 1) * D].astype(bf16)),
                "wo": np.ascontiguousarray(Wo[gi * GD:(gi + 1) * GD, :].astype(bf16)),
                "masks": masks,
            })
    res = run_bass_kernel_spmd(nc, in_maps, list(range(8)))
    out = np.zeros((B, S, H), np.float32)
    for b in range(B):
        acc = None
        for gi in range(KV_HEADS):
            o = res.results[b * KV_HEADS + gi]["out"].astype(np.float32)
            acc = o if acc is None else acc + o
        out[b] = acc.transpose(0, 2, 1, 3).reshape(S, H)              # [16,4,128,512] -> [S,H]
    return out
